# revision 10
# baseline (speedup 1.0000x reference)
"""Trainium2 Bass SPMD kernel: DeepPoly ReLU layer relaxation (N=8192).

Outputs (matching reference): x_out, lower_ret, upper_ret,
lower_weights (NxN diag), upper_weights (NxN diag), lower_bias, upper_bias.

Sharding: neuron dim N split across 8 cores (1024 each). Each core writes its
1024-row slab of both NxN weight matrices. The slab rows are built on-chip as
[128, 8192] tiles that are zero except the diagonal element, placed with a
fused (iota == idx) * val tensor_scalar op where idx comes from a per-core
input tensor -- so the SPMD program is identical on all cores.
"""

import sys

import numpy as np

N = 8192
NCORES = 8
SHARD = N // NCORES  # 1024 neurons per core
TPC = SHARD // 128   # 8 row-tiles of 128 rows per core

_CACHE = {}
TRACE = False
TRACE_KWARGS = {}
LAST_RESULT = None


def _import_concourse():
    try:
        import concourse.bass  # noqa: F401
    except ImportError:
        sys.path.insert(0, "/opt/trn_rl_repo")


def _build():
    _import_concourse()
    import concourse.bacc as bacc
    import concourse.tile as tile
    from concourse import mybir

    f32 = mybir.dt.float32
    op = mybir.AluOpType

    # Bacc (not raw Bass): its compile() splits multi-sem waits into
    # event-semaphore chains -- TRN2 allows at most 1 wait per instruction.
    nc = bacc.Bacc()

    x_in = nc.dram_tensor("x_s", [128, TPC], f32, kind="ExternalInput")
    l_in = nc.dram_tensor("lower_s", [128, TPC], f32, kind="ExternalInput")
    u_in = nc.dram_tensor("upper_s", [128, TPC], f32, kind="ExternalInput")
    i_in = nc.dram_tensor("idx_s", [128, TPC], f32, kind="ExternalInput")
    iota_in = nc.dram_tensor("iota_in", [128, N], f32, kind="ExternalInput")

    uw_out = nc.dram_tensor("uw", [SHARD, N], f32, kind="ExternalOutput")
    lw_out = nc.dram_tensor("lw", [SHARD, N], f32, kind="ExternalOutput")
    sm_out = nc.dram_tensor("small", [128, 5 * TPC], f32, kind="ExternalOutput")

    T = TPC
    with tile.TileContext(nc) as tc:
        with (
            tc.tile_pool(name="singles", bufs=1) as singles,
            tc.tile_pool(name="big", bufs=8) as bigpool,
        ):
            X = singles.tile([128, T], f32)
            L = singles.tile([128, T], f32)
            U = singles.tile([128, T], f32)
            IDX = singles.tile([128, T], f32)
            nc.sync.dma_start(out=U[:], in_=u_in[:, :])
            nc.sync.dma_start(out=L[:], in_=l_in[:, :])
            nc.sync.dma_start(out=IDX[:], in_=i_in[:, :])
            nc.sync.dma_start(out=X[:], in_=x_in[:, :])

            # column index ramp 0..N-1, identical in every partition;
            # loaded as input (two halves on the two HWDGE rings) -- much
            # faster than gpsimd iota and off the critical DVE path
            H = N // 2
            IOTA0 = singles.tile([128, H], f32)
            IOTA1 = singles.tile([128, H], f32)
            nc.sync.dma_start(out=IOTA0[:], in_=iota_in[:, 0:H])
            nc.scalar.dma_start(out=IOTA1[:], in_=iota_in[:, H:N])
            IOTAS = (IOTA0, IOTA1)

            # per-neuron branch math on [128, TPC] tiles
            notneg = singles.tile([128, T], f32)  # u > 0   (== lw_diag)
            nc.vector.tensor_scalar(notneg[:], U[:], 0.0, None, op.is_gt)
            lneg = singles.tile([128, T], f32)  # l < 0
            nc.vector.tensor_scalar(lneg[:], L[:], 0.0, None, op.is_lt)
            cross = singles.tile([128, T], f32)  # crossing branch
            nc.vector.tensor_mul(cross[:], notneg[:], lneg[:])
            d = singles.tile([128, T], f32)
            nc.vector.tensor_sub(d[:], U[:], L[:])
            # clamp away from 0 so recip stays finite; exact where cross=1
            # (there d = u-l > 0), and the clamped lanes are masked by cross
            dsafe = singles.tile([128, T], f32)
            nc.vector.tensor_scalar_max(dsafe[:], d[:], 1e-30)
            r = singles.tile([128, T], f32)
            nc.vector.reciprocal(r[:], dsafe[:])
            slope = singles.tile([128, T], f32)
            nc.vector.tensor_mul(slope[:], U[:], r[:])
            slopec = singles.tile([128, T], f32)  # cross * slope
            nc.vector.tensor_mul(slopec[:], cross[:], slope[:])
            pos = singles.tile([128, T], f32)  # l >= 0
            nc.vector.tensor_scalar(pos[:], L[:], 0.0, None, op.is_ge)
            tmp = singles.tile([128, T], f32)
            nc.vector.tensor_add(tmp[:], pos[:], slopec[:])
            uwd = singles.tile([128, T], f32)  # uw_diag
            nc.vector.tensor_mul(uwd[:], tmp[:], notneg[:])

            # packed small outputs: x_out | lower_ret | upper_ret | upper_bias | lower_bias
            sm = singles.tile([128, 5 * T], f32)
            nc.vector.tensor_relu(sm[:, 0:T], X[:])
            nc.vector.tensor_mul(sm[:, T : 2 * T], L[:], notneg[:])
            nc.vector.tensor_mul(sm[:, 2 * T : 3 * T], U[:], uwd[:])
            nc.vector.tensor_mul(sm[:, 3 * T : 4 * T], slopec[:], L[:])
            nc.vector.memset(sm[:, 4 * T : 5 * T], 0.0)
            nc.sync.dma_start(out=sm_out[:, :], in_=sm[:])

            # weight slabs: one [128, N/2] source half-tile per
            # (matrix, row-tile, half), zero except the diagonal element
            # per row (placed by the fused is_equal*mult against iota)
            ring = 0
            for t in range(T):
                for wout, val in ((uw_out, uwd), (lw_out, notneg)):
                    for h in range(2):
                        big = bigpool.tile([128, H], f32)
                        nc.vector.tensor_scalar(
                            big[:],
                            IOTAS[h][:],
                            IDX[:, t : t + 1],
                            val[:, t : t + 1],
                            op.is_equal,
                            op.mult,
                        )
                        eng = nc.sync if ring % 2 == 0 else nc.scalar
                        ring += 1
                        eng.dma_start(
                            out=wout[t * 128 : (t + 1) * 128, h * H : (h + 1) * H],
                            in_=big[:],
                        )
    nc.compile()
    return nc


def _shard2d(v):
    # (1024,) -> [128, TPC] with (p, t) holding element t*128+p
    return np.ascontiguousarray(v.reshape(TPC, 128).T.astype(np.float32))


_IOTA_INPUT = np.ascontiguousarray(
    np.tile(np.arange(N, dtype=np.float32), (128, 1))
)


def kernel(x, lower, upper, input_shape=None, **_unused):
    global LAST_RESULT
    _import_concourse()
    from concourse import bass_utils

    x = np.asarray(x, dtype=np.float32).reshape(N)
    lower = np.asarray(lower, dtype=np.float32).reshape(N)
    upper = np.asarray(upper, dtype=np.float32).reshape(N)

    if "nc" not in _CACHE:
        _CACHE["nc"] = _build()
    nc = _CACHE["nc"]

    in_maps = []
    for c in range(NCORES):
        sl = slice(c * SHARD, (c + 1) * SHARD)
        in_maps.append(
            {
                "x_s": _shard2d(x[sl]),
                "lower_s": _shard2d(lower[sl]),
                "upper_s": _shard2d(upper[sl]),
                "idx_s": _shard2d(
                    np.arange(c * SHARD, (c + 1) * SHARD, dtype=np.float32)
                ),
                "iota_in": _IOTA_INPUT,
            }
        )

    res = bass_utils.run_bass_kernel_spmd(
        nc,
        in_maps,
        core_ids=list(range(NCORES)),
        trace=TRACE,
        **TRACE_KWARGS,
    )
    LAST_RESULT = res
    cores = res.results

    upper_weights = np.concatenate([r["uw"] for r in cores], axis=0)
    lower_weights = np.concatenate([r["lw"] for r in cores], axis=0)

    def unpack(col):
        parts = [
            cores[c]["small"][:, col * TPC : (col + 1) * TPC].T.reshape(-1)
            for c in range(NCORES)
        ]
        return np.concatenate(parts).reshape(1, N)

    x_out = unpack(0)
    lower_ret = unpack(1)
    upper_ret = unpack(2)
    upper_bias = unpack(3)
    lower_bias = unpack(4)

    return (
        x_out,
        lower_ret,
        upper_ret,
        lower_weights,
        upper_weights,
        lower_bias,
        upper_bias,
    )


# revision 13
# speedup vs baseline: 1.0341x; 1.0341x over previous
"""Trainium2 Bass SPMD kernel: DeepPoly ReLU layer relaxation (N=8192).

Outputs (matching reference): x_out, lower_ret, upper_ret,
lower_weights (NxN diag), upper_weights (NxN diag), lower_bias, upper_bias.

Sharding: neuron dim N split across 8 cores (1024 each). Each core writes its
1024-row slab of both NxN weight matrices. The slab rows are built on-chip as
[128, 8192] tiles that are zero except the diagonal element, placed with a
fused (iota == idx) * val tensor_scalar op where idx comes from a per-core
input tensor -- so the SPMD program is identical on all cores.
"""

import sys

import numpy as np

N = 8192
NCORES = 8
SHARD = N // NCORES  # 1024 neurons per core
TPC = SHARD // 128   # 8 row-tiles of 128 rows per core

_CACHE = {}
TRACE = False
TRACE_KWARGS = {}
LAST_RESULT = None


def _import_concourse():
    try:
        import concourse.bass  # noqa: F401
    except ImportError:
        sys.path.insert(0, "/opt/trn_rl_repo")


def _build():
    _import_concourse()
    import concourse.bacc as bacc
    import concourse.tile as tile
    from concourse import mybir

    f32 = mybir.dt.float32
    op = mybir.AluOpType

    # Bacc (not raw Bass): its compile() splits multi-sem waits into
    # event-semaphore chains -- TRN2 allows at most 1 wait per instruction.
    nc = bacc.Bacc()

    # packed per-core vectors: cols [0:T)=x  [T:2T)=lower  [2T:3T)=upper
    # [3T:4T)=idx -- one DMA instead of four 32B-descriptor sprays
    vin = nc.dram_tensor("vin", [128, 4 * TPC], f32, kind="ExternalInput")
    iota_in = nc.dram_tensor("iota_in", [128, N], f32, kind="ExternalInput")

    uw_out = nc.dram_tensor("uw", [SHARD, N], f32, kind="ExternalOutput")
    lw_out = nc.dram_tensor("lw", [SHARD, N], f32, kind="ExternalOutput")
    sm_out = nc.dram_tensor("small", [128, 5 * TPC], f32, kind="ExternalOutput")

    T = TPC
    with tile.TileContext(nc) as tc:
        with (
            tc.tile_pool(name="singles", bufs=1) as singles,
            tc.tile_pool(name="big", bufs=8) as bigpool,
        ):
            V = singles.tile([128, 4 * T], f32)
            nc.scalar.dma_start(out=V[:], in_=vin[:, :])
            X = V[:, 0:T]
            L = V[:, T : 2 * T]
            U = V[:, 2 * T : 3 * T]
            IDX = V[:, 3 * T : 4 * T]

            # column index ramp 0..N-1, identical in every partition;
            # loaded as input (two halves on the two HWDGE rings) -- much
            # faster than gpsimd iota and off the critical DVE path
            H = N // 2
            IOTA0 = singles.tile([128, H], f32)
            IOTA1 = singles.tile([128, H], f32)
            nc.sync.dma_start(out=IOTA0[:], in_=iota_in[:, 0:H])
            nc.scalar.dma_start(out=IOTA1[:], in_=iota_in[:, H:N])
            IOTAS = (IOTA0, IOTA1)

            # per-neuron branch math on [128, TPC] tiles
            notneg = singles.tile([128, T], f32)  # u > 0   (== lw_diag)
            nc.vector.tensor_scalar(notneg[:], U[:], 0.0, None, op.is_gt)
            lneg = singles.tile([128, T], f32)  # l < 0
            nc.vector.tensor_scalar(lneg[:], L[:], 0.0, None, op.is_lt)
            cross = singles.tile([128, T], f32)  # crossing branch
            nc.vector.tensor_mul(cross[:], notneg[:], lneg[:])
            d = singles.tile([128, T], f32)
            nc.vector.tensor_sub(d[:], U[:], L[:])
            # clamp away from 0 so recip stays finite; exact where cross=1
            # (there d = u-l > 0), and the clamped lanes are masked by cross
            dsafe = singles.tile([128, T], f32)
            nc.vector.tensor_scalar_max(dsafe[:], d[:], 1e-30)
            r = singles.tile([128, T], f32)
            nc.vector.reciprocal(r[:], dsafe[:])
            slope = singles.tile([128, T], f32)
            nc.vector.tensor_mul(slope[:], U[:], r[:])
            slopec = singles.tile([128, T], f32)  # cross * slope
            nc.vector.tensor_mul(slopec[:], cross[:], slope[:])
            pos = singles.tile([128, T], f32)  # l >= 0
            nc.vector.tensor_scalar(pos[:], L[:], 0.0, None, op.is_ge)
            tmp = singles.tile([128, T], f32)
            nc.vector.tensor_add(tmp[:], pos[:], slopec[:])
            uwd = singles.tile([128, T], f32)  # uw_diag
            nc.vector.tensor_mul(uwd[:], tmp[:], notneg[:])

            # packed small outputs: x_out | lower_ret | upper_ret | upper_bias | lower_bias
            sm = singles.tile([128, 5 * T], f32)
            nc.vector.tensor_relu(sm[:, 0:T], X[:])
            nc.vector.tensor_mul(sm[:, T : 2 * T], L[:], notneg[:])
            nc.vector.tensor_mul(sm[:, 2 * T : 3 * T], U[:], uwd[:])
            nc.vector.tensor_mul(sm[:, 3 * T : 4 * T], slopec[:], L[:])
            nc.vector.memset(sm[:, 4 * T : 5 * T], 0.0)
            nc.sync.dma_start(out=sm_out[:, :], in_=sm[:])

            # weight slabs: one [128, N/2] source half-tile per
            # (matrix, row-tile, half), zero except the diagonal element
            # per row (placed by the fused is_equal*mult against iota)
            ring = 0
            for t in range(T):
                for wout, val in ((uw_out, uwd), (lw_out, notneg)):
                    for h in range(2):
                        big = bigpool.tile([128, H], f32)
                        nc.vector.tensor_scalar(
                            big[:],
                            IOTAS[h][:],
                            IDX[:, t : t + 1],
                            val[:, t : t + 1],
                            op.is_equal,
                            op.mult,
                        )
                        eng = nc.sync if ring % 2 == 0 else nc.scalar
                        ring += 1
                        eng.dma_start(
                            out=wout[t * 128 : (t + 1) * 128, h * H : (h + 1) * H],
                            in_=big[:],
                        )
    nc.compile()
    return nc


def _shard2d(v):
    # (1024,) -> [128, TPC] with (p, t) holding element t*128+p
    return np.ascontiguousarray(v.reshape(TPC, 128).T.astype(np.float32))


_IOTA_INPUT = np.ascontiguousarray(
    np.tile(np.arange(N, dtype=np.float32), (128, 1))
)


def kernel(x, lower, upper, input_shape=None, **_unused):
    global LAST_RESULT
    _import_concourse()
    from concourse import bass_utils

    x = np.asarray(x, dtype=np.float32).reshape(N)
    lower = np.asarray(lower, dtype=np.float32).reshape(N)
    upper = np.asarray(upper, dtype=np.float32).reshape(N)

    if "nc" not in _CACHE:
        _CACHE["nc"] = _build()
    nc = _CACHE["nc"]

    in_maps = []
    for c in range(NCORES):
        sl = slice(c * SHARD, (c + 1) * SHARD)
        vin = np.concatenate(
            [
                _shard2d(x[sl]),
                _shard2d(lower[sl]),
                _shard2d(upper[sl]),
                _shard2d(np.arange(c * SHARD, (c + 1) * SHARD, dtype=np.float32)),
            ],
            axis=1,
        )
        in_maps.append(
            {
                "vin": np.ascontiguousarray(vin),
                "iota_in": _IOTA_INPUT,
            }
        )

    res = bass_utils.run_bass_kernel_spmd(
        nc,
        in_maps,
        core_ids=list(range(NCORES)),
        trace=TRACE,
        **TRACE_KWARGS,
    )
    LAST_RESULT = res
    cores = res.results

    upper_weights = np.concatenate([r["uw"] for r in cores], axis=0)
    lower_weights = np.concatenate([r["lw"] for r in cores], axis=0)

    def unpack(col):
        parts = [
            cores[c]["small"][:, col * TPC : (col + 1) * TPC].T.reshape(-1)
            for c in range(NCORES)
        ]
        return np.concatenate(parts).reshape(1, N)

    x_out = unpack(0)
    lower_ret = unpack(1)
    upper_ret = unpack(2)
    upper_bias = unpack(3)
    lower_bias = unpack(4)

    return (
        x_out,
        lower_ret,
        upper_ret,
        lower_weights,
        upper_weights,
        lower_bias,
        upper_bias,
    )


# revision 42
# speedup vs baseline: 1.0490x; 1.0144x over previous
"""Trainium2 Bass SPMD kernel: DeepPoly ReLU layer relaxation (N=8192).

Outputs (matching reference): x_out, lower_ret, upper_ret,
lower_weights (NxN diag), upper_weights (NxN diag), lower_bias, upper_bias.

Sharding: neuron dim N split across 8 cores (1024 each). Each core writes its
1024-row slab of both NxN weight matrices. The slab rows are built on-chip as
[128, 4096] half-tiles that are zero except the diagonal element, placed with
a fused (iota == idx) * val tensor_scalar op where idx comes from a per-core
input tensor -- so the SPMD program is identical on all cores and all 512 MB
of output is produced on-device at the SDMA/HBM write roofline.
"""

import sys

import numpy as np

N = 8192
NCORES = 8
SHARD = N // NCORES  # 1024 neurons per core
TPC = SHARD // 128   # 8 row-tiles of 128 rows per core

_CACHE = {}
TRACE = False
TRACE_KWARGS = {}
LAST_RESULT = None


def _import_concourse():
    try:
        import concourse.bass  # noqa: F401
    except ImportError:
        sys.path.insert(0, "/opt/trn_rl_repo")


def _build():
    _import_concourse()
    import concourse.bacc as bacc
    import concourse.tile as tile
    from concourse import mybir

    f32 = mybir.dt.float32
    op = mybir.AluOpType

    # Bacc (not raw Bass): its compile() splits multi-sem waits into
    # event-semaphore chains -- TRN2 allows at most 1 wait per instruction.
    nc = bacc.Bacc(enable_partition_id=False)

    # packed per-core vectors: cols [0:T)=x  [T:2T)=lower  [2T:3T)=upper
    # [3T:4T)=idx -- one DMA instead of four 32B-descriptor sprays
    vin = nc.dram_tensor("vin", [128, 4 * TPC], f32, kind="ExternalInput")

    uw_out = nc.dram_tensor("uw", [SHARD, N], f32, kind="ExternalOutput")
    lw_out = nc.dram_tensor("lw", [SHARD, N], f32, kind="ExternalOutput")
    sm_out = nc.dram_tensor("small", [128, 5 * TPC], f32, kind="ExternalOutput")

    T = TPC
    with tile.TileContext(nc) as tc:
        with (
            tc.tile_pool(name="singles", bufs=1) as singles,
            tc.tile_pool(name="big", bufs=10) as bigpool,
        ):
            # half-width column ramp 0..4095, identical in every partition;
            # generated on gpsimd (Pool engine) so no SDMA traffic is spent
            # on it.  The upper output half reuses the same ramp compared
            # against idx-4096, halving iota generation time (which gates
            # the whole DVE stream at startup via the drain event-sem).
            H = N // 2
            IOTA = singles.tile([128, H], f32)
            nc.gpsimd.iota(
                IOTA[:],
                pattern=[[1, H]],
                base=0,
                channel_multiplier=0,
                allow_small_or_imprecise_dtypes=True,
            )

            V = singles.tile([128, 4 * T], f32)
            nc.scalar.dma_start(out=V[:], in_=vin[:, :])
            X = V[:, 0:T]
            L = V[:, T : 2 * T]
            U = V[:, 2 * T : 3 * T]
            IDX = V[:, 3 * T : 4 * T]

            def branch_math(pool, Lap, Uap, P, W, tag):
                """DeepPoly branch math on [P, W] tiles.
                Returns (notneg, slopec, uwd): lw_diag = notneg,
                uw_diag = uwd = (pos + cross*slope) * notneg."""

                names = [
                    "notneg", "lneg", "cross", "d", "dsafe",
                    "r", "slope", "slopec", "pos", "uwd",
                ]
                ts = [
                    pool.tile([P, W], f32, tag=tag + nm, name=tag + nm)
                    for nm in names
                ]
                (notneg, lneg, cross, d, dsafe, r, slope, slopec, pos, uwd) = ts
                nc.vector.tensor_scalar(notneg[:], Uap, 0.0, None, op.is_gt)
                nc.vector.tensor_scalar(lneg[:], Lap, 0.0, None, op.is_lt)
                nc.vector.tensor_mul(cross[:], notneg[:], lneg[:])
                nc.vector.tensor_sub(d[:], Uap, Lap)
                # clamp away from 0 so recip stays finite; exact where
                # cross=1 (there d = u-l > 0); clamped lanes masked by cross
                nc.vector.tensor_scalar_max(dsafe[:], d[:], 1e-30)
                nc.vector.reciprocal(r[:], dsafe[:])
                nc.vector.tensor_mul(slope[:], Uap, r[:])
                nc.vector.tensor_mul(slopec[:], cross[:], slope[:])
                nc.vector.tensor_scalar(pos[:], Lap, 0.0, None, op.is_ge)
                nc.vector.tensor_add(uwd[:], pos[:], slopec[:])
                nc.vector.tensor_mul(uwd[:], uwd[:], notneg[:])
                return notneg, slopec, uwd

            notneg, slopec, uwd = branch_math(
                singles, L[:], U[:], 128, T, "a"
            )
            IDX2 = singles.tile([128, T], f32)  # idx - 4096 for upper half
            nc.vector.tensor_scalar_add(IDX2[:], IDX[:], -float(H))

            # packed small outputs: x_out | lower_ret | upper_ret | upper_bias | lower_bias
            sm = singles.tile([128, 5 * T], f32)
            nc.vector.tensor_relu(sm[:, 0:T], X[:])
            nc.vector.tensor_mul(sm[:, T : 2 * T], L[:], notneg[:])
            nc.vector.tensor_mul(sm[:, 2 * T : 3 * T], U[:], uwd[:])
            nc.vector.tensor_mul(sm[:, 3 * T : 4 * T], slopec[:], L[:])
            nc.vector.memset(sm[:, 4 * T : 5 * T], 0.0)
            nc.sync.dma_start(out=sm_out[:, :], in_=sm[:])

            # weight slabs: one [128, W] source tile per
            # (matrix, row-tile, column-chunk), zero except the diagonal
            # element per row (placed by the fused is_equal*mult against
            # iota).  NOTE: sources must span all 128 partitions --
            # partial-partition DMAs collapse the 16-engine descriptor
            # swizzle onto 4 engines.
            # 16KB-per-partition descriptors (4096-wide halves) hit the
            # ~27 GB/s per-engine line rate; smaller chunks drop it ~18%.
            ring = 0
            for t in range(T):
                for wout, val in ((uw_out, uwd), (lw_out, notneg)):
                    for h, idx in ((0, IDX), (1, IDX2)):
                        big = bigpool.tile([128, H], f32, name="big", tag="big")
                        nc.vector.tensor_scalar(
                            big[:, :],
                            IOTA[:, :],
                            idx[:, t : t + 1],
                            val[:, t : t + 1],
                            op.is_equal,
                            op.mult,
                        )
                        eng = nc.sync if ring % 2 == 0 else nc.scalar
                        ring += 1
                        eng.dma_start(
                            out=wout[
                                t * 128 : (t + 1) * 128, h * H : (h + 1) * H
                            ],
                            in_=big[:, :],
                        )

    nc.compile()
    return nc


def _shard2d(v):
    # (1024,) -> [128, TPC] with (p, t) holding element t*128+p
    return np.ascontiguousarray(v.reshape(TPC, 128).T.astype(np.float32))


def _core_inputs(c, x, lower, upper):
    sl = slice(c * SHARD, (c + 1) * SHARD)
    vin = np.concatenate(
        [
            _shard2d(x[sl]),
            _shard2d(lower[sl]),
            _shard2d(upper[sl]),
            _shard2d(np.arange(c * SHARD, (c + 1) * SHARD, dtype=np.float32)),
        ],
        axis=1,
    )
    return {"vin": np.ascontiguousarray(vin)}


def kernel(x, lower, upper, input_shape=None, **_unused):
    global LAST_RESULT
    _import_concourse()
    from concourse import bass_utils

    x = np.asarray(x, dtype=np.float32).reshape(N)
    lower = np.asarray(lower, dtype=np.float32).reshape(N)
    upper = np.asarray(upper, dtype=np.float32).reshape(N)

    if "nc" not in _CACHE:
        _CACHE["nc"] = _build()
    nc = _CACHE["nc"]

    in_maps = [_core_inputs(c, x, lower, upper) for c in range(NCORES)]

    res = bass_utils.run_bass_kernel_spmd(
        nc,
        in_maps,
        core_ids=list(range(NCORES)),
        trace=TRACE,
        **TRACE_KWARGS,
    )
    LAST_RESULT = res
    cores = res.results

    upper_weights = np.concatenate([r["uw"] for r in cores], axis=0)
    lower_weights = np.concatenate([r["lw"] for r in cores], axis=0)

    def unpack(col):
        parts = [
            cores[c]["small"][:, col * TPC : (col + 1) * TPC].T.reshape(-1)
            for c in range(NCORES)
        ]
        return np.concatenate(parts).reshape(1, N)

    x_out = unpack(0)
    lower_ret = unpack(1)
    upper_ret = unpack(2)
    upper_bias = unpack(3)
    lower_bias = unpack(4)

    return (
        x_out,
        lower_ret,
        upper_ret,
        lower_weights,
        upper_weights,
        lower_bias,
        upper_bias,
    )

